# revision 26
# baseline (speedup 1.0000x reference)
"""Expert-parallel MoE routing kernel for Trainium2 (8 NeuronCores).

Problem: top-k(=2) softmax-gated MoE FFN (relu), followed by
log_softmax(sum(moe_out, axis=-1)) over the sequence dim.

Key algebraic observation: the graded output is
    log_softmax_S( sum_d moe_out[t, d] )
and
    sum_d moe_out[t, :] = sum_e g[t,e] * (relu(x_t @ W1_e + b1_e) @ rowsum(W2_e) + sum(b2_e))
so the entire second expert matmul collapses to a matvec against
s_e = rowsum(W2_e), which the host computes once (a single pass over w2);
w2 (8 MB/core in the baseline) never touches the device.  The gate values
are folded into the dispatched tokens on the host (g * relu(x@W1) ==
relu((g*x)@W1) for g > 0 when b1 == 0; with b1 != 0 the gate is applied
on the host instead).  HBM traffic per core: ~5.4 MB (bf16 w1 + tokens).

Per-core device program (core e owns expert e):
  PE : h_pre = xtg^T @ W1  (bf16 operands, 256 matmuls of ~276 cols,
       which stream at the warm-clock floor of ~120 ns/MM)
  ACT: relu(h_pre + b1) per [128, ln] m-tile, PSUM -> SBUF
  DVE: acc += relu_h * s_m  (signed per-partition scale, fused mult-add)
  PE : z = ones^T @ acc     (final 128-partition reduction)
Host gathers z per core, scatter-adds into [T], applies log_softmax.

Timing notes (measured): ~7.2 us fixed prologue + ~9.5 us fixed ucode
teardown bracket the kernel; a scratch-data PE warmup during the
prologue flips the HAM clock gate to 2.4 GHz before the real stream;
DMAs are few and big (each dma_start costs ~650 ns serial issue +
~2 us completion latency, and one HWDGE ring drains FIFO, so the
critical pieces go first: tokens, then w1 group-by-group).
"""

import os

import numpy as np

N_CORES = 8
P = 128
GRP = 2  # m-tiles per w1 column-group (one 256-col block per (group, kd))
NWARM = int(os.environ.get("MOE_NWARM", "13"))
USE_Q = os.environ.get("MOE_Q", "1") == "1"  # fp8 head-start (chunk0, g<NQ)
NQ = int(os.environ.get("MOE_NQ", "3"))  # head-start groups (fp8 supply chain)
SX = 16.0    # power-of-2 scale for fp8 tokens (xg max ~5 -> ~80, e4m3 max 240)
SW = 512.0   # power-of-2 scale for fp8 head-start weights (w1 max ~0.18 -> ~92)


def _round_up(v, m):
    return ((v + m - 1) // m) * m


_BUILD_CACHE = {}


def _grp_for(NC):
    # chunk-phase-major loop: grp accumulators + 1 warmup tile per group in
    # flight, independent of NC — GRP=2 always fits the 6-buf PSUM pool
    return GRP


def _build_program(D, H, ln, NC, use_q):
    """Trace + compile the single-core program (SPMD across 8 cores).

    Per-core inputs:
      xtg [P, NC*KD*ln] bf16  gate-scaled gathered tokens; block (c, kd) at
                              cols (c*KD+kd)*ln is xg[kd*P:(kd+1)*P, c*ln:(c+1)*ln]
      w1  [P, KD*H]     bf16  expert's first-layer weight; block (g, kd) at
                              cols (g*KD+kd)*GP is w1[kd*P:(kd+1)*P, g*GP:(g+1)*GP]
      sm  [P, 2*MH+1+grp] f32 cols [0:MH) = w2 row-sums (col m = s[m*P:(m+1)*P]),
                              [MH:2MH) = b1 tiled the same way, [2MH] = ones,
                              [2MH+1:2MH+1+grp] = w2 row-sums for g0's m-tiles
                              divided by SX*SW (fp8 head-start descale)
      xq  [P, KD*ln]    fp8e4 chunk-0 tokens * SX   (only when use_q)
      wq  [P, KD*GP]    fp8e4 g0 weights * SW       (only when use_q)
    Output:
      z [1, NC*ln] f32  z[c] = sum_h s_h * relu(x_c @ w1_h + b1_h)

    With use_q, (chunk 0, group 0) matmuls run on the fp8 copies, whose DMA
    pieces are 544 KB total vs the 1.09 MB bf16 critical path — the real
    stream starts ~9.4us instead of ~13.1us.  The fp8 error affects 1/8 of
    the h-units of half the tokens (~9e-3 rel, budget 2e-2).
    """
    key = (D, H, ln, NC, use_q)
    if key in _BUILD_CACHE:
        return _BUILD_CACHE[key]

    import concourse.tile as tile
    from concourse import bacc, mybir

    f32 = mybir.dt.float32
    f32r = mybir.dt.float32r
    bf16 = mybir.dt.bfloat16
    fp8 = mybir.dt.float8e4
    KD = D // P   # k-tiles over D
    MH = H // P   # m-tiles over H
    grp = _grp_for(NC)
    NG = MH // grp  # w1 column groups
    GP = grp * P  # columns per w1 block
    nq = min(NQ, NG - 1) if use_q else 0

    nc = bacc.Bacc("TRN2", target_bir_lowering=False, debug=False)
    xtg_d = nc.dram_tensor("xtg", [P, KD * NC * ln], bf16, kind="ExternalInput").ap()
    w1_d = nc.dram_tensor("w1", [P, KD * H], bf16, kind="ExternalInput").ap()
    sm_d = nc.dram_tensor(
        "sm", [P, 2 * MH + 1 + max(1, nq) * grp], f32, kind="ExternalInput"
    ).ap()
    if use_q:
        xq_d = nc.dram_tensor("xq", [P, KD * ln], fp8, kind="ExternalInput").ap()
        wq_d = nc.dram_tensor("wq", [P, nq * KD * GP], fp8, kind="ExternalInput").ap()
    z_d = nc.dram_tensor("z", [1, NC * ln], f32, kind="ExternalOutput").ap()

    with tile.TileContext(nc) as tc:
        with (
            tc.tile_pool(name="persist", bufs=1) as persist,
            tc.tile_pool(name="ht", bufs=6) as htp,
            tc.tile_pool(name="psum_h", bufs=6, space="PSUM") as psum_h,
            tc.tile_pool(name="psum_z", bufs=2, space="PSUM") as psum_z,
        ):
            # --- small loads first: w2 row-sums / b1 / ones ---
            # (smalls ride the ACT HWDGE ring so they never queue behind the
            # bulk xtg/w1 stream on the SP ring)
            sm_sb = persist.tile([P, 2 * MH + 1 + max(1, nq) * grp], f32)
            nc.scalar.dma_start(out=sm_sb[:], in_=sm_d[:])
            w2s = sm_sb[:, 0:MH]
            b1t = sm_sb[:, MH : 2 * MH]
            w2s_q = sm_sb[:, 2 * MH + 1 :]  # descaled rowsums for fp8 m-tiles
            ones = persist.tile([P, 1], f32r)
            nc.vector.tensor_copy(out=ones[:], in_=sm_sb[:, 2 * MH : 2 * MH + 1])
            # f32r copy of the last group's w2 row-sums: the last group skips
            # the DVE chain and matvecs relu output directly on the PE, which
            # needs f32r operands (plain f32 engages the 4x-slow fp32 mode)
            w2sr = persist.tile([P, grp], f32r)
            nc.vector.tensor_copy(out=w2sr[:], in_=sm_sb[:, MH - grp : MH])

            # acc tiles — f32r so the final PE matvec accepts them; two
            # independent accumulation chains (even/odd m) per chunk halve
            # the serial DVE tail.  g == 0 writes them fresh (no memset:
            # walrus rejects f32r memset).
            acc = [
                [
                    persist.tile([P, ln], f32r, tag=f"acc{c}_{p}", name=f"acc{c}_{p}")
                    for p in range(grp)
                ]
                for c in range(NC)
            ] if NG > 1 else None

            # --- weights + activations ---
            xtg_sb = persist.tile([P, KD * NC * ln], bf16)
            w1_sb = persist.tile([P, KD * H], bf16)
            if use_q:
                xq_sb = persist.tile([P, KD * ln], fp8)
                wq_sb = persist.tile([P, nq * KD * GP], fp8)

            def xtg_load(c, kd0, nkd, eng=None):
                sl = slice((c * KD + kd0) * ln, (c * KD + kd0 + nkd) * ln)
                (eng or nc.sync).dma_start(out=xtg_sb[:, sl], in_=xtg_d[:, sl])

            def w1_load(g, kd0, nkd, eng=None):
                sl = slice((g * KD + kd0) * GP, (g * KD + kd0 + nkd) * GP)
                (eng or nc.sync).dma_start(out=w1_sb[:, sl], in_=w1_d[:, sl])

            # PE warmup: ~4.6us of dependency-free matmuls on scratch data
            # run during the fixed ~7us program prologue, flipping the HAM
            # clock-gate to 2.4GHz before the real stream starts (saves the
            # ~4us half-clock ramp the stream would otherwise pay)
            if NWARM:
                warm = persist.tile([P, ln], bf16, tag="warm", name="warm")
                nc.vector.memset(warm[:], 0.0)
                pwarm = psum_h.tile([P, ln], f32, tag="psh", name="psh")
                for i in range(NWARM):
                    nc.tensor.matmul(
                        pwarm[:],
                        warm[:, 0:P],
                        warm[:],
                        start=(i == 0),
                        stop=(i == NWARM - 1),
                        skip_group_check=True,
                    )

            # each dma_start costs ~650ns of serial HWDGE issue time and
            # ~1.5-2us completion latency, and queued transfers drain FIFO —
            # few big pieces, most-critical first, ALL on the SP ring.
            # (Measured dead ends: kd-splitting the first pieces recycles the
            # 8 DMAHW sem lanes and the first matmul inherits a false
            # dependency on an unrelated later DMA; the ACT HWDGE ring posts
            # completions ~6-9us late even for small transfers, so bulk loads
            # there stall the PE and re-throttle the HAM.)  Phase A (chunk 0)
            # is gated by xtg chunk 0 + w1 g0 only; later chunks' xtg arrives
            # while phase A computes.
            if use_q:
                # fp8 head-start supply chain: chunk-0 groups 0..nq-1 run on
                # fp8 copies whose pieces (282 + nq*262 KB) post early enough
                # to keep the cold-clock PE fed until the bf16 stream catches
                # up; the bf16 copies of those groups (needed only by later
                # chunks) stream last.
                qeng = nc.gpsimd if os.environ.get("MOE_QDMA", "swdge") == "swdge" else nc.sync
                qeng.dma_start(out=xq_sb[:], in_=xq_d[:])
                for q in range(nq):
                    sl = slice(q * KD * GP, (q + 1) * KD * GP)
                    qeng.dma_start(out=wq_sb[:, sl], in_=wq_d[:, sl])
                xtg_load(0, 0, KD)
                for c in range(1, NC):
                    xtg_load(c, 0, KD)
                for g in range(nq, NG):
                    w1_load(g, 0, KD)
                for g in range(nq):
                    w1_load(g, 0, KD)
            else:
                xtg_load(0, 0, KD)
                for g in range(NG):
                    w1_load(g, 0, KD)
                for c in range(1, NC):
                    xtg_load(c, 0, KD)

            # --- mm1 + relu + scaled accumulate, chunk-phase major: the
            # whole w1 stream paces phase A (16 MMs/group vs ~3.4us/group
            # DMA), and later phases run with zero DMA dependencies while
            # earlier chunks' reduce/copy/store overlap them ---
            z_sb = persist.tile([1, NC * ln], f32)
            for c in range(NC):
                pz = psum_z.tile([1, ln], f32, tag="psz", name="psz")
                for g in range(NG):
                    last_g = g == NG - 1
                    pss = [
                        psum_h.tile([P, ln], f32, tag="psh", name="psh")
                        for _ in range(grp)
                    ]
                    q_mm = use_q and c == 0 and g < nq
                    for kd in range(KD):
                        base = (g * KD + kd) * GP
                        for mi in range(grp):
                            if q_mm:
                                qb = (g * KD + kd) * GP
                                lhsT = wq_sb[:, qb + mi * P : qb + (mi + 1) * P]
                                rhs = xq_sb[:, kd * ln : (kd + 1) * ln]
                            else:
                                lhsT = w1_sb[:, base + mi * P : base + (mi + 1) * P]
                                rhs = xtg_sb[
                                    :, (c * KD + kd) * ln : (c * KD + kd + 1) * ln
                                ]
                            nc.tensor.matmul(
                                pss[mi][:],
                                lhsT,
                                rhs,
                                start=(kd == 0),
                                stop=(kd == KD - 1),
                                skip_group_check=True,
                            )
                    if last_g and NG > 1:
                        # the m0..m(NG*grp-3) chains closed during this
                        # group's matmuls — reduce them into pz now, so only
                        # this group's relu outputs remain on the tail
                        for p in range(grp):
                            nc.tensor.matmul(
                                pz[:],
                                ones[:],
                                acc[c][p][:],
                                start=(p == 0),
                                stop=False,
                                skip_group_check=True,
                            )
                    for mi in range(grp):
                        m = g * grp + mi
                        ht = htp.tile([P, ln], f32r, tag="ht", name="ht")
                        if last_g and mi > 0:
                            # the tail's relus serialize on ACT; run this one
                            # on the (idle) DVE in parallel: max(pre + b1, 0)
                            nc.vector.tensor_scalar(
                                out=ht[:],
                                in0=pss[mi][:],
                                scalar1=b1t[:, m : m + 1],
                                scalar2=0.0,
                                op0=mybir.AluOpType.add,
                                op1=mybir.AluOpType.max,
                            )
                        else:
                            nc.scalar.activation(
                                ht[:],
                                pss[mi][:],
                                mybir.ActivationFunctionType.Relu,
                                bias=b1t[:, m : m + 1],
                            )
                        if last_g:
                            # bypass the DVE chain: scale+reduce this m-tile
                            # directly on the PE (w2s column as stationary)
                            nc.tensor.matmul(
                                pz[:],
                                w2sr[:, mi : mi + 1],
                                ht[:],
                                start=(NG == 1 and mi == 0),
                                stop=(mi == grp - 1),
                                skip_group_check=True,
                            )
                        elif g == 0:
                            sc = w2s_q[:, m : m + 1] if q_mm else w2s[:, m : m + 1]
                            nc.vector.tensor_scalar(
                                out=acc[c][mi][:],
                                in0=ht[:],
                                scalar1=sc,
                                scalar2=None,
                                op0=mybir.AluOpType.mult,
                            )
                        else:
                            sc = w2s_q[:, m : m + 1] if q_mm else w2s[:, m : m + 1]
                            nc.vector.scalar_tensor_tensor(
                                out=acc[c][mi][:],
                                in0=ht[:],
                                scalar=sc,
                                in1=acc[c][mi][:],
                                op0=mybir.AluOpType.mult,
                                op1=mybir.AluOpType.add,
                            )

                # phase-end store for this chunk (for all but the last chunk
                # this hides under the next phase)
                sl = slice(c * ln, (c + 1) * ln)
                nc.scalar.activation(
                    z_sb[:, sl],
                    pz[:],
                    mybir.ActivationFunctionType.Copy,
                    bias=0.0,
                )
                nc.sync.dma_start(out=z_d[:, sl], in_=z_sb[:, sl])

    nc.compile()
    _BUILD_CACHE[key] = nc
    return nc


def kernel(x, wg, w1, b1, w2, b2, k):
    import ml_dtypes
    from concourse.bass_utils import run_bass_kernel_spmd

    bf16 = ml_dtypes.bfloat16
    x = np.asarray(x)
    wg = np.asarray(wg)
    w1 = np.asarray(w1)
    b1 = np.asarray(b1)
    w2 = np.asarray(w2)
    b2 = np.asarray(b2)
    k = int(k)

    B, S, D = x.shape
    E = wg.shape[1]
    H = w1.shape[2]
    T = B * S
    KD = D // P
    MH = H // P
    assert E == N_CORES, f"expert-parallel layout assumes E == 8, got {E}"

    xf = np.ascontiguousarray(x.reshape(T, D), dtype=np.float32)

    # --- gate + top-k routing (host; needed to build the dispatch shards) ---
    logits = xf @ wg.astype(np.float32)
    logits -= logits.max(axis=1, keepdims=True)
    np.exp(logits, out=logits)
    scores = logits / logits.sum(axis=1, keepdims=True)
    if k >= E:
        topi = np.broadcast_to(np.arange(E, dtype=np.int64), (T, E))
    else:
        topi = np.argpartition(-scores, k, axis=1)[:, :k]
    rows = np.arange(T)[:, None]
    topv = scores[rows, topi]

    # per-expert token lists
    idx_e = []
    val_e = []
    for e in range(E):
        tmask, kpos = np.nonzero(topi == e)
        idx_e.append(tmask)
        val_e.append(topv[tmask, kpos].astype(np.float32))
    max_cnt = max(len(i) for i in idx_e)

    # chunk geometry: NC chunks of ln <= 512 tokens (PSUM bank limit)
    NC = max(1, -(-max_cnt // 512))
    ln = _round_up(-(-max_cnt // NC), 4)
    C = NC * ln
    grp = _grp_for(NC)
    assert D % P == 0 and H % P == 0 and MH % grp == 0, (D, H)

    fold_gate = not b1.any()  # g*relu(u) == relu(g*u) only when b1 == 0
    use_q = USE_Q and fold_gate  # fp8 head-start needs b1 == 0 (scale folding)
    s_e = w2.astype(np.float32).sum(axis=2)          # [E, H] row-sums
    b2s_e = b2.astype(np.float32).sum(axis=1)        # [E]

    grp = _grp_for(NC)
    NG = MH // grp
    nq = min(NQ, NG - 1) if use_q else 0
    fp8np = ml_dtypes.float8_e4m3  # TRN fp8_e4-compatible bits for |v| <= 240

    nc = _build_program(D, H, ln, NC, use_q)

    in_maps = []
    for e in range(E):
        n_e = len(idx_e[e])
        xg = np.zeros((D, C), dtype=np.float32)
        if n_e:
            cols = xf[idx_e[e]].T
            if fold_gate:
                cols = cols * val_e[e][None, :]
            xg[:, :n_e] = cols
        # -> [P, NC, KD, ln]
        xg4 = xg.reshape(KD, P, NC, ln).transpose(1, 2, 0, 3)
        xtg = np.ascontiguousarray(xg4.reshape(P, NC * KD * ln)).astype(bf16)
        # w1 -> [P, KD*H], block (g, kd) of grp*P columns
        w14 = (
            w1[e]
            .astype(np.float32)
            .reshape(KD, P, MH // grp, grp * P)
            .transpose(1, 2, 0, 3)
        )  # [P, NG, KD, GP]
        w1r = np.ascontiguousarray(w14.reshape(P, KD * H)).astype(bf16)
        sm = np.empty((P, 2 * MH + 1 + max(1, nq) * grp), dtype=np.float32)
        sm[:, 0:MH] = s_e[e].reshape(MH, P).T
        sm[:, MH : 2 * MH] = b1[e].astype(np.float32).reshape(MH, P).T
        sm[:, 2 * MH] = 1.0
        sm[:, 2 * MH + 1 :] = sm[:, 0 : max(1, nq) * grp] / (SX * SW)
        m = {"xtg": xtg, "w1": w1r, "sm": sm}
        if use_q:
            xq = np.clip(xg4[:, 0] * SX, -240.0, 240.0)      # [P, KD, ln]
            wq = np.clip(w14[:, :nq] * SW, -240.0, 240.0)    # [P, nq, KD, GP]
            m["xq"] = np.ascontiguousarray(xq.reshape(P, -1)).astype(fp8np)
            m["wq"] = np.ascontiguousarray(wq.reshape(P, -1)).astype(fp8np)
        in_maps.append(m)

    res = run_bass_kernel_spmd(nc, in_maps, core_ids=list(range(N_CORES)))

    # --- combine: scatter-add per-(token, expert) scalars, then log_softmax ---
    s = np.zeros(T, dtype=np.float32)
    for e in range(E):
        n_e = len(idx_e[e])
        if n_e:
            z = res.results[e]["z"][0, :n_e].astype(np.float32)
            if fold_gate:
                s[idx_e[e]] += z
            else:
                s[idx_e[e]] += val_e[e] * z
    if b2s_e.any():
        for e in range(E):
            if len(idx_e[e]):
                s[idx_e[e]] += val_e[e] * b2s_e[e]

    sm = s.reshape(B, S)
    sm = sm - sm.max(axis=1, keepdims=True)
    out = sm - np.log(np.exp(sm).sum(axis=1, keepdims=True))
    return out.astype(np.float32)



# revision 30
# speedup vs baseline: 1.2079x; 1.2079x over previous
"""Expert-parallel MoE routing kernel for Trainium2 (8 NeuronCores).

Problem: top-k(=2) softmax-gated MoE FFN (relu), followed by
log_softmax(sum(moe_out, axis=-1)) over the sequence dim.

Key algebraic observation: the graded output is
    log_softmax_S( sum_d moe_out[t, d] )
and
    sum_d moe_out[t, :] = sum_e g[t,e] * (relu(x_t @ W1_e + b1_e) @ rowsum(W2_e) + sum(b2_e))
so the entire second expert matmul collapses to a matvec against
s_e = rowsum(W2_e), which the host computes once (a single pass over w2);
w2 (8 MB/core in the baseline) never touches the device.  The gate values
are folded into the dispatched tokens on the host (g * relu(x@W1) ==
relu((g*x)@W1) for g > 0 when b1 == 0; with b1 != 0 the gate is applied
on the host instead).  HBM traffic per core: ~5.4 MB (bf16 w1 + tokens).

Per-core device program (core e owns expert e):
  PE : h_pre = xtg^T @ W1  (bf16 operands, 256 matmuls of ~276 cols,
       which stream at the warm-clock floor of ~120 ns/MM)
  ACT: relu(h_pre + b1) per [128, ln] m-tile, PSUM -> SBUF
  DVE: acc += relu_h * s_m  (signed per-partition scale, fused mult-add)
  PE : z = ones^T @ acc     (final 128-partition reduction)
Host gathers z per core, scatter-adds into [T], applies log_softmax.

Timing notes (measured): ~7.2 us fixed prologue + ~9.5 us fixed ucode
teardown bracket the kernel; a scratch-data PE warmup during the
prologue flips the HAM clock gate to 2.4 GHz before the real stream;
DMAs are few and big (each dma_start costs ~650 ns serial issue +
~2 us completion latency, and one HWDGE ring drains FIFO, so the
critical pieces go first: tokens, then w1 group-by-group).
"""

import os

import numpy as np

N_CORES = 8
P = 128
GRP = 2  # m-tiles per w1 column-group (one 256-col block per (group, kd))
NWARM = int(os.environ.get("MOE_NWARM", "16"))
USE_Q = os.environ.get("MOE_Q", "1") == "1"  # fp8 head-start (chunk0, g<NQ)
NQ = int(os.environ.get("MOE_NQ", "3"))  # head-start groups (fp8 supply chain)
SX = 16.0    # power-of-2 scale for fp8 tokens (xg max ~5 -> ~80, e4m3 max 240)
SW = 512.0   # power-of-2 scale for fp8 head-start weights (w1 max ~0.18 -> ~92)


def _round_up(v, m):
    return ((v + m - 1) // m) * m


_BUILD_CACHE = {}


def _grp_for(NC):
    # chunk-phase-major loop: grp accumulators + 1 warmup tile per group in
    # flight, independent of NC — GRP=2 always fits the 6-buf PSUM pool
    return GRP


def _build_program(D, H, ln, NC, use_q):
    """Trace + compile the single-core program (SPMD across 8 cores).

    Per-core inputs:
      xtg [P, NC*KD*ln] bf16  gate-scaled gathered tokens; block (c, kd) at
                              cols (c*KD+kd)*ln is xg[kd*P:(kd+1)*P, c*ln:(c+1)*ln]
      w1  [P, KD*H]     bf16  expert's first-layer weight; block (g, kd) at
                              cols (g*KD+kd)*GP is w1[kd*P:(kd+1)*P, g*GP:(g+1)*GP]
      sm  [P, 2*MH+1+grp] f32 cols [0:MH) = w2 row-sums (col m = s[m*P:(m+1)*P]),
                              [MH:2MH) = b1 tiled the same way, [2MH] = ones,
                              [2MH+1:2MH+1+grp] = w2 row-sums for g0's m-tiles
                              divided by SX*SW (fp8 head-start descale)
      xq  [P, KD*ln]    fp8e4 chunk-0 tokens * SX   (only when use_q)
      wq  [P, KD*GP]    fp8e4 g0 weights * SW       (only when use_q)
    Output:
      z [1, NC*ln] f32  z[c] = sum_h s_h * relu(x_c @ w1_h + b1_h)

    With use_q, (chunk 0, group 0) matmuls run on the fp8 copies, whose DMA
    pieces are 544 KB total vs the 1.09 MB bf16 critical path — the real
    stream starts ~9.4us instead of ~13.1us.  The fp8 error affects 1/8 of
    the h-units of half the tokens (~9e-3 rel, budget 2e-2).
    """
    key = (D, H, ln, NC, use_q)
    if key in _BUILD_CACHE:
        return _BUILD_CACHE[key]

    import concourse.tile as tile
    from concourse import bacc, mybir

    f32 = mybir.dt.float32
    f32r = mybir.dt.float32r
    bf16 = mybir.dt.bfloat16
    fp8 = mybir.dt.float8e4
    KD = D // P   # k-tiles over D
    MH = H // P   # m-tiles over H
    grp = _grp_for(NC)
    NG = MH // grp  # w1 column groups
    GP = grp * P  # columns per w1 block
    nq = min(NQ, NG - 1) if use_q else 0

    nc = bacc.Bacc("TRN2", target_bir_lowering=False, debug=False)
    xtg_d = nc.dram_tensor("xtg", [P, KD * NC * ln], bf16, kind="ExternalInput").ap()
    w1_d = nc.dram_tensor("w1", [P, KD * H], bf16, kind="ExternalInput").ap()
    sm_d = nc.dram_tensor(
        "sm", [P, 2 * MH + 1 + max(1, nq) * grp], f32, kind="ExternalInput"
    ).ap()
    if use_q:
        xq_d = nc.dram_tensor("xq", [P, KD * ln], fp8, kind="ExternalInput").ap()
        wq_d = nc.dram_tensor("wq", [P, nq * KD * GP], fp8, kind="ExternalInput").ap()
    z_d = nc.dram_tensor("z", [1, NC * ln], f32, kind="ExternalOutput").ap()

    with tile.TileContext(nc) as tc:
        with (
            tc.tile_pool(name="persist", bufs=1) as persist,
            tc.tile_pool(name="ht", bufs=6) as htp,
            tc.tile_pool(name="psum_h", bufs=6, space="PSUM") as psum_h,
            tc.tile_pool(name="psum_z", bufs=2, space="PSUM") as psum_z,
        ):
            # --- small loads first: w2 row-sums / b1 / ones ---
            # (smalls ride the ACT HWDGE ring so they never queue behind the
            # bulk xtg/w1 stream on the SP ring)
            sm_sb = persist.tile([P, 2 * MH + 1 + max(1, nq) * grp], f32)
            nc.scalar.dma_start(out=sm_sb[:], in_=sm_d[:])
            w2s = sm_sb[:, 0:MH]
            b1t = sm_sb[:, MH : 2 * MH]
            w2s_q = sm_sb[:, 2 * MH + 1 :]  # descaled rowsums for fp8 m-tiles
            ones = persist.tile([P, 1], f32r)
            nc.vector.tensor_copy(out=ones[:], in_=sm_sb[:, 2 * MH : 2 * MH + 1])
            # f32r copy of the last group's w2 row-sums: the last group skips
            # the DVE chain and matvecs relu output directly on the PE, which
            # needs f32r operands (plain f32 engages the 4x-slow fp32 mode)
            w2sr = persist.tile([P, grp], f32r)
            nc.vector.tensor_copy(out=w2sr[:], in_=sm_sb[:, MH - grp : MH])

            # acc tiles — f32r so the final PE matvec accepts them; two
            # independent accumulation chains (even/odd m) per chunk halve
            # the serial DVE tail.  g == 0 writes them fresh (no memset:
            # walrus rejects f32r memset).
            acc = [
                [
                    persist.tile([P, ln], f32r, tag=f"acc{c}_{p}", name=f"acc{c}_{p}")
                    for p in range(grp)
                ]
                for c in range(NC)
            ] if NG > 1 else None

            # tiny dummy DMA heads the SP ring: the FIRST piece of an
            # execution pays an inflated, jittery completion receipt
            # (measured 2.6-4.6us vs ~2.2 steady-state); burning it on 128
            # bytes nobody consumes pulls the first real piece's post time
            # in by up to ~2us on unlucky cores.
            dummy_sb = persist.tile([1, 16], f32)
            nc.sync.dma_start(out=dummy_sb[:], in_=sm_d[0:1, 0:16])

            # --- weights + activations ---
            xtg_sb = persist.tile([P, KD * NC * ln], bf16)
            w1_sb = persist.tile([P, KD * H], bf16)
            if use_q:
                xq_sb = persist.tile([P, KD * ln], fp8)
                wq_sb = persist.tile([P, nq * KD * GP], fp8)

            def xtg_load(c, kd0, nkd, eng=None):
                sl = slice((c * KD + kd0) * ln, (c * KD + kd0 + nkd) * ln)
                (eng or nc.sync).dma_start(out=xtg_sb[:, sl], in_=xtg_d[:, sl])

            def w1_load(g, kd0, nkd, eng=None):
                sl = slice((g * KD + kd0) * GP, (g * KD + kd0 + nkd) * GP)
                (eng or nc.sync).dma_start(out=w1_sb[:, sl], in_=w1_d[:, sl])

            # PE warmup: ~4.6us of dependency-free matmuls on scratch data
            # run during the fixed ~7us program prologue, flipping the HAM
            # clock-gate to 2.4GHz before the real stream starts (saves the
            # ~4us half-clock ramp the stream would otherwise pay)
            if NWARM:
                warm = persist.tile([P, ln], bf16, tag="warm", name="warm")
                nc.vector.memset(warm[:], 0.0)
                pwarm = psum_h.tile([P, ln], f32, tag="psh", name="psh")
                for i in range(NWARM):
                    nc.tensor.matmul(
                        pwarm[:],
                        warm[:, 0:P],
                        warm[:],
                        start=(i == 0),
                        stop=(i == NWARM - 1),
                        skip_group_check=True,
                    )

            # each dma_start costs ~650ns of serial HWDGE issue time and
            # ~1.5-2us completion latency, and queued transfers drain FIFO —
            # few big pieces, most-critical first, ALL on the SP ring.
            # (Measured dead ends: kd-splitting the first pieces recycles the
            # 8 DMAHW sem lanes and the first matmul inherits a false
            # dependency on an unrelated later DMA; the ACT HWDGE ring posts
            # completions ~6-9us late even for small transfers, so bulk loads
            # there stall the PE and re-throttle the HAM.)  Phase A (chunk 0)
            # is gated by xtg chunk 0 + w1 g0 only; later chunks' xtg arrives
            # while phase A computes.
            if use_q:
                # fp8 head-start supply chain: chunk-0 groups 0..nq-1 run on
                # fp8 copies whose pieces (282 + nq*262 KB) post early enough
                # to keep the cold-clock PE fed until the bf16 stream catches
                # up; the bf16 copies of those groups (needed only by later
                # chunks) stream last.
                nc.sync.dma_start(out=xq_sb[:], in_=xq_d[:])
                for q in range(nq):
                    sl = slice(q * KD * GP, (q + 1) * KD * GP)
                    nc.sync.dma_start(out=wq_sb[:, sl], in_=wq_d[:, sl])
                xtg_load(0, 0, KD)
                for c in range(1, NC):
                    xtg_load(c, 0, KD)
                for g in range(nq, NG):
                    w1_load(g, 0, KD)
                for g in range(nq):
                    w1_load(g, 0, KD)
            else:
                xtg_load(0, 0, KD)
                for g in range(NG):
                    w1_load(g, 0, KD)
                for c in range(1, NC):
                    xtg_load(c, 0, KD)

            # --- mm1 + relu + scaled accumulate, chunk-phase major: the
            # whole w1 stream paces phase A (16 MMs/group vs ~3.4us/group
            # DMA), and later phases run with zero DMA dependencies while
            # earlier chunks' reduce/copy/store overlap them ---
            z_sb = persist.tile([1, NC * ln], f32)
            for c in range(NC):
                pz = psum_z.tile([1, ln], f32, tag="psz", name="psz")
                for g in range(NG):
                    last_g = g == NG - 1
                    pss = [
                        psum_h.tile([P, ln], f32, tag="psh", name="psh")
                        for _ in range(grp)
                    ]
                    q_mm = use_q and c == 0 and g < nq
                    for kd in range(KD):
                        base = (g * KD + kd) * GP
                        for mi in range(grp):
                            if q_mm:
                                qb = (g * KD + kd) * GP
                                lhsT = wq_sb[:, qb + mi * P : qb + (mi + 1) * P]
                                rhs = xq_sb[:, kd * ln : (kd + 1) * ln]
                            else:
                                lhsT = w1_sb[:, base + mi * P : base + (mi + 1) * P]
                                rhs = xtg_sb[
                                    :, (c * KD + kd) * ln : (c * KD + kd + 1) * ln
                                ]
                            nc.tensor.matmul(
                                pss[mi][:],
                                lhsT,
                                rhs,
                                start=(kd == 0),
                                stop=(kd == KD - 1),
                                skip_group_check=True,
                            )
                    if last_g and NG > 1:
                        # the m0..m(NG*grp-3) chains closed during this
                        # group's matmuls — reduce them into pz now, so only
                        # this group's relu outputs remain on the tail
                        for p in range(grp):
                            nc.tensor.matmul(
                                pz[:],
                                ones[:],
                                acc[c][p][:],
                                start=(p == 0),
                                stop=False,
                                skip_group_check=True,
                            )
                    for mi in range(grp):
                        m = g * grp + mi
                        ht = htp.tile([P, ln], f32r, tag="ht", name="ht")
                        if last_g and mi > 0:
                            # the tail's relus serialize on ACT; run this one
                            # on the (idle) DVE in parallel: max(pre + b1, 0)
                            nc.vector.tensor_scalar(
                                out=ht[:],
                                in0=pss[mi][:],
                                scalar1=b1t[:, m : m + 1],
                                scalar2=0.0,
                                op0=mybir.AluOpType.add,
                                op1=mybir.AluOpType.max,
                            )
                        else:
                            nc.scalar.activation(
                                ht[:],
                                pss[mi][:],
                                mybir.ActivationFunctionType.Relu,
                                bias=b1t[:, m : m + 1],
                            )
                        if last_g:
                            # bypass the DVE chain: scale+reduce this m-tile
                            # directly on the PE (w2s column as stationary)
                            nc.tensor.matmul(
                                pz[:],
                                w2sr[:, mi : mi + 1],
                                ht[:],
                                start=(NG == 1 and mi == 0),
                                stop=(mi == grp - 1),
                                skip_group_check=True,
                            )
                        elif g == 0:
                            sc = w2s_q[:, m : m + 1] if q_mm else w2s[:, m : m + 1]
                            nc.vector.tensor_scalar(
                                out=acc[c][mi][:],
                                in0=ht[:],
                                scalar1=sc,
                                scalar2=None,
                                op0=mybir.AluOpType.mult,
                            )
                        else:
                            sc = w2s_q[:, m : m + 1] if q_mm else w2s[:, m : m + 1]
                            nc.vector.scalar_tensor_tensor(
                                out=acc[c][mi][:],
                                in0=ht[:],
                                scalar=sc,
                                in1=acc[c][mi][:],
                                op0=mybir.AluOpType.mult,
                                op1=mybir.AluOpType.add,
                            )

                # phase-end store for this chunk (for all but the last chunk
                # this hides under the next phase)
                sl = slice(c * ln, (c + 1) * ln)
                nc.scalar.activation(
                    z_sb[:, sl],
                    pz[:],
                    mybir.ActivationFunctionType.Copy,
                    bias=0.0,
                )
                nc.sync.dma_start(out=z_d[:, sl], in_=z_sb[:, sl])

    nc.compile()
    _BUILD_CACHE[key] = nc
    return nc


def kernel(x, wg, w1, b1, w2, b2, k):
    import ml_dtypes
    from concourse.bass_utils import run_bass_kernel_spmd

    bf16 = ml_dtypes.bfloat16
    x = np.asarray(x)
    wg = np.asarray(wg)
    w1 = np.asarray(w1)
    b1 = np.asarray(b1)
    w2 = np.asarray(w2)
    b2 = np.asarray(b2)
    k = int(k)

    B, S, D = x.shape
    E = wg.shape[1]
    H = w1.shape[2]
    T = B * S
    KD = D // P
    MH = H // P
    assert E == N_CORES, f"expert-parallel layout assumes E == 8, got {E}"

    xf = np.ascontiguousarray(x.reshape(T, D), dtype=np.float32)

    # --- gate + top-k routing (host; needed to build the dispatch shards) ---
    logits = xf @ wg.astype(np.float32)
    logits -= logits.max(axis=1, keepdims=True)
    np.exp(logits, out=logits)
    scores = logits / logits.sum(axis=1, keepdims=True)
    if k >= E:
        topi = np.broadcast_to(np.arange(E, dtype=np.int64), (T, E))
    else:
        topi = np.argpartition(-scores, k, axis=1)[:, :k]
    rows = np.arange(T)[:, None]
    topv = scores[rows, topi]

    # per-expert token lists
    idx_e = []
    val_e = []
    for e in range(E):
        tmask, kpos = np.nonzero(topi == e)
        idx_e.append(tmask)
        val_e.append(topv[tmask, kpos].astype(np.float32))
    max_cnt = max(len(i) for i in idx_e)

    # chunk geometry: NC chunks of ln <= 512 tokens (PSUM bank limit)
    NC = max(1, -(-max_cnt // 512))
    ln = _round_up(-(-max_cnt // NC), 4)
    C = NC * ln
    grp = _grp_for(NC)
    assert D % P == 0 and H % P == 0 and MH % grp == 0, (D, H)

    fold_gate = not b1.any()  # g*relu(u) == relu(g*u) only when b1 == 0
    use_q = USE_Q and fold_gate  # fp8 head-start needs b1 == 0 (scale folding)
    s_e = w2.astype(np.float32).sum(axis=2)          # [E, H] row-sums
    b2s_e = b2.astype(np.float32).sum(axis=1)        # [E]

    grp = _grp_for(NC)
    NG = MH // grp
    nq = min(NQ, NG - 1) if use_q else 0
    fp8np = ml_dtypes.float8_e4m3  # TRN fp8_e4-compatible bits for |v| <= 240

    nc = _build_program(D, H, ln, NC, use_q)

    in_maps = []
    for e in range(E):
        n_e = len(idx_e[e])
        xg = np.zeros((D, C), dtype=np.float32)
        if n_e:
            cols = xf[idx_e[e]].T
            if fold_gate:
                cols = cols * val_e[e][None, :]
            xg[:, :n_e] = cols
        # -> [P, NC, KD, ln]
        xg4 = xg.reshape(KD, P, NC, ln).transpose(1, 2, 0, 3)
        xtg = np.ascontiguousarray(xg4.reshape(P, NC * KD * ln)).astype(bf16)
        # w1 -> [P, KD*H], block (g, kd) of grp*P columns
        w14 = (
            w1[e]
            .astype(np.float32)
            .reshape(KD, P, MH // grp, grp * P)
            .transpose(1, 2, 0, 3)
        )  # [P, NG, KD, GP]
        w1r = np.ascontiguousarray(w14.reshape(P, KD * H)).astype(bf16)
        sm = np.empty((P, 2 * MH + 1 + max(1, nq) * grp), dtype=np.float32)
        sm[:, 0:MH] = s_e[e].reshape(MH, P).T
        sm[:, MH : 2 * MH] = b1[e].astype(np.float32).reshape(MH, P).T
        sm[:, 2 * MH] = 1.0
        sm[:, 2 * MH + 1 :] = sm[:, 0 : max(1, nq) * grp] / (SX * SW)
        m = {"xtg": xtg, "w1": w1r, "sm": sm}
        if use_q:
            xq = np.clip(xg4[:, 0] * SX, -240.0, 240.0)      # [P, KD, ln]
            wq = np.clip(w14[:, :nq] * SW, -240.0, 240.0)    # [P, nq, KD, GP]
            m["xq"] = np.ascontiguousarray(xq.reshape(P, -1)).astype(fp8np)
            m["wq"] = np.ascontiguousarray(wq.reshape(P, -1)).astype(fp8np)
        in_maps.append(m)

    res = run_bass_kernel_spmd(nc, in_maps, core_ids=list(range(N_CORES)))

    # --- combine: scatter-add per-(token, expert) scalars, then log_softmax ---
    s = np.zeros(T, dtype=np.float32)
    for e in range(E):
        n_e = len(idx_e[e])
        if n_e:
            z = res.results[e]["z"][0, :n_e].astype(np.float32)
            if fold_gate:
                s[idx_e[e]] += z
            else:
                s[idx_e[e]] += val_e[e] * z
    if b2s_e.any():
        for e in range(E):
            if len(idx_e[e]):
                s[idx_e[e]] += val_e[e] * b2s_e[e]

    sm = s.reshape(B, S)
    sm = sm - sm.max(axis=1, keepdims=True)
    out = sm - np.log(np.exp(sm).sum(axis=1, keepdims=True))
    return out.astype(np.float32)

